# revision 26
# baseline (speedup 1.0000x reference)
"""NT-Xent loss kernel for 8 Trainium2 NeuronCores (Bass/Tile).

Strategy (data-parallel rows, SPMD):
  - Host: concat z_i,z_j -> reps [8192, 512], cast bf16. Core c receives
    np.roll(reps, -c*1024, axis=0) so every core runs the same static
    program on "its" first 1024 rows: self-similarity for local row li
    sits at column li, the positive partner at column li+4096.
  - On-chip per core: normalize rows (f32 stats, bf16 data), transpose via
    PE into repsT [D, N] (bf16), then the [1024, 8192] block of the
    similarity matrix as 128x1024 PSUM tiles (bf16 matmul, f32 accum).
    Self column is masked with a -1e30 eye tile; per tile the row-max runs
    on DVE and exp(4*sim-4) row-sums are fused into one ScalarE
    activation+accum.
  - Host: combine per-core stats (positives, hardest negatives, exp sums)
    in float64 into the scalar loss (the two "all-reduced" loss terms).
"""

import numpy as np
import ml_dtypes

import concourse.bacc as bacc
import concourse.bass as bass
import concourse.tile as tile
import concourse.mybir as mybir
from concourse.bass_utils import run_bass_kernel_spmd

B = 4096
D = 512
N = 2 * B            # 8192 rows total
NCORES = 8
NLOC = N // NCORES   # 1024 rows per core
RT = N // 128        # 64 row tiles
MT = NLOC // 128     # 8 local row tiles
NNW = 1024           # column super-tile width (2 PSUM banks)
NN = N // NNW        # 8 column super-tiles
KT = D // 128        # 4 contraction chunks

F32 = mybir.dt.float32
I32 = mybir.dt.int32
BF16 = mybir.dt.bfloat16

_CACHE = {}


def _build_program():
    if "nc" in _CACHE:
        return _CACHE["nc"]
    nc = bacc.Bacc(
        "TRN2",
        target_bir_lowering=False,
        debug=False,
        num_devices=NCORES,
    )

    z = nc.dram_tensor("z", [N, D], BF16, kind="ExternalInput").ap()
    ident = nc.dram_tensor("ident", [128, 128], BF16, kind="ExternalInput").ap()
    negeye = nc.dram_tensor("negeye", [128, 128], F32, kind="ExternalInput").ap()

    maxc_d = nc.dram_tensor("maxc", [MT, 128, NN], F32, kind="ExternalOutput").ap()
    esum_d = nc.dram_tensor("esum", [MT, 128, NN], F32, kind="ExternalOutput").ap()
    posd_d = nc.dram_tensor("posd", [128, MT], F32, kind="ExternalOutput").ap()
    invn_d = nc.dram_tensor("invn", [128, RT], F32, kind="ExternalOutput").ap()

    AX = mybir.AxisListType
    ALU = mybir.AluOpType
    AF = mybir.ActivationFunctionType

    with tile.TileContext(nc) as tc:
        with (
            tc.tile_pool(name="persist", bufs=1) as persist,
            tc.tile_pool(name="nrows", bufs=3) as nrows,
            tc.tile_pool(name="trash", bufs=2) as trashp,
            tc.tile_pool(name="etrash", bufs=2) as etrashp,
            tc.tile_pool(name="pstr", bufs=2, space="PSUM") as pstrp,
            tc.tile_pool(name="mm", bufs=3, space="PSUM") as mmp,
        ):
            zfull = persist.tile([128, RT, 512], BF16, tag="zfull")
            repsT = persist.tile([128, KT, N], BF16, tag="repsT")
            identS = persist.tile([128, 128], BF16, tag="identS")
            negeyeS = persist.tile([128, 128], F32, tag="negeyeS")
            ssqall = persist.tile([128, RT], F32, tag="ssqall")
            invall = persist.tile([128, RT], F32, tag="invall")
            posdt = persist.tile([128, MT], F32, tag="posdt")
            negfour = persist.tile([128, 1], F32, tag="negfour")

            nc.vector.memset(negfour, -4.0)
            nc.sync.dma_start(out=identS, in_=ident)
            nc.sync.dma_start(out=negeyeS, in_=negeye)

            # per-m stat accumulators: column g holds stats of the g-th
            # 1024-wide column super-tile. maxm holds max of exp(4*sim-4)
            # (bf16, from the exp tile); host takes log to recover sim max.
            maxm = [
                persist.tile([128, NN], F32, tag=f"maxm{m}", name=f"maxm{m}")
                for m in range(MT)
            ]
            esm = [
                persist.tile([128, NN], F32, tag=f"esm{m}", name=f"esm{m}")
                for m in range(MT)
            ]

            def prep_dma(g):
                for r in range(g * 8, g * 8 + 8):
                    nc.sync.dma_start(
                        out=zfull[:, r, :], in_=z[r * 128 : (r + 1) * 128, :]
                    )

            def prep_load(g):
                """squared norms + inv + normalized rows for group g."""
                for r in range(g * 8, g * 8 + 8):
                    tr = trashp.tile([128, NNW], BF16, tag="trash")
                    nc.scalar.activation(
                        out=tr[:, :512],
                        in_=zfull[:, r, :],
                        func=AF.Square,
                        accum_out=ssqall[:, r : r + 1],
                    )
                gs = slice(g * 8, g * 8 + 8)
                # rsqrt on DVE (Quake seed + 2 Newton steps) — keeps
                # ScalarE on the exp/square table set only (no Sqrt-set
                # table reloads interleaved with the main-loop Exp)
                ry = trashp.tile([128, 8], F32, tag="rsq_y")
                rh = trashp.tile([128, 8], F32, tag="rsq_h")
                rt = trashp.tile([128, 8], F32, tag="rsq_t")
                yi = ry.bitcast(I32)
                nc.vector.tensor_scalar(
                    out=yi, in0=ssqall[:, gs].bitcast(I32), scalar1=1,
                    scalar2=None, op0=ALU.logical_shift_right,
                )
                nc.vector.tensor_scalar(
                    out=yi, in0=yi, scalar1=1597463007, scalar2=-1,
                    op0=ALU.subtract, op1=ALU.mult,
                )
                nc.vector.tensor_scalar_mul(rh, ssqall[:, gs], 0.5)
                for step in range(2):
                    nc.vector.tensor_mul(rt, ry, ry)
                    nc.vector.tensor_mul(rt, rt, rh)
                    nc.vector.tensor_scalar(
                        out=rt, in0=rt, scalar1=-1.0, scalar2=1.5,
                        op0=ALU.mult, op1=ALU.add,
                    )
                    if step == 0:
                        nc.vector.tensor_mul(ry, ry, rt)
                    else:
                        nc.vector.tensor_mul(invall[:, gs], ry, rt)
                for r in range(g * 8, g * 8 + 8):
                    nrow = nrows.tile([128, 512], BF16, tag="nrow")
                    nc.vector.tensor_scalar_mul(
                        nrow, zfull[:, r, :], invall[:, r : r + 1]
                    )
                    yield r, nrow

            def prep_transpose(r, nrow):
                pstr = pstrp.tile([128, KT, 128], BF16, tag="pstr")
                for k in range(KT):
                    nc.tensor.transpose(
                        pstr[:, k, :], nrow[:, k * 128 : (k + 1) * 128], identS
                    )
                nc.vector.tensor_copy(
                    out=repsT[:, :, r * 128 : (r + 1) * 128], in_=pstr
                )

            def main_group(g):
                """column super-tile g of the sim block, all m."""
                for m in range(MT):
                    ps = mmp.tile([128, NNW], F32, tag="ps")
                    for h in (0, 1):
                        for k in range(KT):
                            nc.tensor.matmul(
                                ps[:, h * 512 : (h + 1) * 512],
                                lhsT=repsT[:, k, m * 128 : (m + 1) * 128],
                                rhs=repsT[
                                    :, k, g * NNW + h * 512 : g * NNW + (h + 1) * 512
                                ],
                                start=(k == 0),
                                stop=(k == KT - 1),
                            )
                    if g == 0:
                        # mask self-similarity: sim[p, m*128+p] -= 1e30
                        nc.vector.tensor_add(
                            ps[:, m * 128 : (m + 1) * 128],
                            ps[:, m * 128 : (m + 1) * 128],
                            negeyeS,
                        )
                    et = etrashp.tile([128, NNW], BF16, tag="etrash")
                    nc.scalar.activation(
                        out=et,
                        in_=ps,
                        func=AF.Exp,
                        bias=negfour,
                        scale=4.0,
                        accum_out=esm[m][:, g : g + 1],
                    )
                    nc.vector.reduce_max(maxm[m][:, g : g + 1], et, axis=AX.X)
                # positives (raw bf16 dots of rolled rows q, q+32), two per
                # group once their partner group is resident
                if g >= 4:
                    for q in (2 * (g - 4), 2 * (g - 4) + 1):
                        prod = nrows.tile([128, 512], F32, tag="prod")
                        nc.vector.tensor_mul(
                            prod, zfull[:, q, :], zfull[:, q + 32, :]
                        )
                        nc.vector.reduce_sum(posdt[:, q : q + 1], prod, axis=AX.X)

            # ---- software-pipelined schedule: prep one group ahead,
            # DMA two groups ahead ----
            prep_dma(0)
            prep_dma(1)
            for r, nrow in prep_load(0):
                prep_transpose(r, nrow)
            for g in range(NN):
                if g + 2 < NN:
                    prep_dma(g + 2)
                pending = list(prep_load(g + 1)) if g + 1 < NN else []
                main_group(g)
                for r, nrow in pending:
                    prep_transpose(r, nrow)

            for m in range(MT):
                nc.sync.dma_start(out=maxc_d[m], in_=maxm[m])
                nc.sync.dma_start(out=esum_d[m], in_=esm[m])
            nc.sync.dma_start(out=posd_d, in_=posdt)
            nc.sync.dma_start(out=invn_d, in_=invall)

    nc.compile()
    _CACHE["nc"] = nc
    return nc


def _host_inputs(z_i, z_j):
    reps = np.concatenate(
        [np.asarray(z_i, np.float32), np.asarray(z_j, np.float32)], axis=0
    )
    zb = reps.astype(ml_dtypes.bfloat16)
    ident = np.eye(128, dtype=np.float32).astype(ml_dtypes.bfloat16)
    negeye = (np.eye(128, dtype=np.float32) * -1.0e30).astype(np.float32)
    in_maps = []
    for c in range(NCORES):
        zc = np.ascontiguousarray(np.roll(zb, -c * NLOC, axis=0))
        in_maps.append({"z": zc, "ident": ident, "negeye": negeye})
    return in_maps


def _combine(results):
    pos = np.zeros(N, np.float64)
    hn = np.zeros(N, np.float64)
    S = 0.0
    for c, o in enumerate(results):
        maxc = np.asarray(o["maxc"], np.float64)   # [MT, 128, NN]
        esum = np.asarray(o["esum"], np.float64)   # [MT, 128, NN]
        posd = np.asarray(o["posd"], np.float64)   # [128, MT]
        invn = np.asarray(o["invn"], np.float64)   # [128, RT]
        # maxc holds max over exp(4*sim-4) per column super-tile (bf16
        # rounded); invert the exp to recover the sim max.
        hn_loc = (np.log(maxc.max(axis=2).reshape(NLOC)) + 4.0) / 4.0
        S += esum.sum()                            # self terms exp'd to 0
        invrow = invn.T.reshape(N)                 # rolled row index
        posl = posd.T.reshape(NLOC) * invrow[:NLOC] * invrow[B : B + NLOC]
        gl = (np.arange(NLOC) + c * NLOC) % N
        pos[gl] = posl
        hn[gl] = hn_loc
    ce = np.mean(np.logaddexp(0.0, 40.0 * hn - 20.0 * pos))
    npairs = N * (N - 1) // 2
    uniformity = np.log(S / 2.0 / npairs)
    return np.array(ce + 0.2 * uniformity, dtype=np.float32)


def run(z_i, z_j, **spmd_kwargs):
    nc = _build_program()
    in_maps = _host_inputs(z_i, z_j)
    res = run_bass_kernel_spmd(nc, in_maps, core_ids=list(range(NCORES)), **spmd_kwargs)
    return _combine(res.results), res


def kernel(z_i, z_j):
    loss, _ = run(z_i, z_j)
    return loss


# revision 28
# speedup vs baseline: 1.0993x; 1.0993x over previous
"""NT-Xent loss kernel for 8 Trainium2 NeuronCores (Bass/Tile).

Strategy (data-parallel rows, SPMD):
  - Host: concat z_i,z_j -> reps [8192, 512], cast bf16. Core c receives
    np.roll(reps, -c*1024, axis=0) so every core runs the same static
    program on "its" first 1024 rows: self-similarity for local row li
    sits at column li, the positive partner at column li+4096.
  - On-chip per core: normalize rows (f32 stats, bf16 data), transpose via
    PE into repsT [D, N] (bf16), then the [1024, 8192] block of the
    similarity matrix as 128x1024 PSUM tiles (bf16 matmul, f32 accum).
    Self column is masked with a -1e30 eye tile; per tile the row-max runs
    on DVE and exp(4*sim-4) row-sums are fused into one ScalarE
    activation+accum.
  - Host: combine per-core stats (positives, hardest negatives, exp sums)
    in float64 into the scalar loss (the two "all-reduced" loss terms).
"""

import numpy as np
import ml_dtypes

import concourse.bacc as bacc
import concourse.bass as bass
import concourse.tile as tile
import concourse.mybir as mybir
from concourse.bass_utils import run_bass_kernel_spmd

B = 4096
D = 512
N = 2 * B            # 8192 rows total
NCORES = 8
NLOC = N // NCORES   # 1024 rows per core
RT = N // 128        # 64 row tiles
MT = NLOC // 128     # 8 local row tiles
NNW = 1024           # column super-tile width (2 PSUM banks)
NN = N // NNW        # 8 column super-tiles
KT = D // 128        # 4 contraction chunks

F32 = mybir.dt.float32
I32 = mybir.dt.int32
BF16 = mybir.dt.bfloat16

_CACHE = {}


def _build_program():
    if "nc" in _CACHE:
        return _CACHE["nc"]
    nc = bacc.Bacc(
        "TRN2",
        target_bir_lowering=False,
        debug=False,
        num_devices=NCORES,
    )

    z = nc.dram_tensor("z", [N, D], BF16, kind="ExternalInput").ap()
    ident = nc.dram_tensor("ident", [128, 128], BF16, kind="ExternalInput").ap()
    negeye = nc.dram_tensor("negeye", [128, 128], F32, kind="ExternalInput").ap()

    maxc_d = nc.dram_tensor("maxc", [MT, 128, NN], F32, kind="ExternalOutput").ap()
    esum_d = nc.dram_tensor("esum", [MT, 128, NN], F32, kind="ExternalOutput").ap()
    posd_d = nc.dram_tensor("posd", [128, MT], F32, kind="ExternalOutput").ap()
    invn_d = nc.dram_tensor("invn", [128, RT], F32, kind="ExternalOutput").ap()

    AX = mybir.AxisListType
    ALU = mybir.AluOpType
    AF = mybir.ActivationFunctionType

    with tile.TileContext(nc) as tc:
        with (
            tc.tile_pool(name="persist", bufs=1) as persist,
            tc.tile_pool(name="nrows", bufs=3) as nrows,
            tc.tile_pool(name="trash", bufs=2) as trashp,
            tc.tile_pool(name="etrash", bufs=2) as etrashp,
            tc.tile_pool(name="pstr", bufs=2, space="PSUM") as pstrp,
            tc.tile_pool(name="mm", bufs=3, space="PSUM") as mmp,
        ):
            zfull = persist.tile([128, RT, 512], BF16, tag="zfull")
            repsT = persist.tile([128, KT, N], BF16, tag="repsT")
            identS = persist.tile([128, 128], BF16, tag="identS")
            negeyeS = persist.tile([128, 128], F32, tag="negeyeS")
            ssqall = persist.tile([128, RT], F32, tag="ssqall")
            nrmall = persist.tile([128, RT], F32, tag="nrmall")
            invall = persist.tile([128, RT], F32, tag="invall")
            posdt = persist.tile([128, MT], F32, tag="posdt")
            negfour = persist.tile([128, 1], F32, tag="negfour")

            nc.vector.memset(negfour, -4.0)
            nc.sync.dma_start(out=identS, in_=ident)
            nc.sync.dma_start(out=negeyeS, in_=negeye)

            # per-m stat accumulators: column g holds stats of the g-th
            # 1024-wide column super-tile. maxm holds max of exp(4*sim-4)
            # (bf16, from the exp tile); host takes log to recover sim max.
            maxm = [
                persist.tile([128, NN], F32, tag=f"maxm{m}", name=f"maxm{m}")
                for m in range(MT)
            ]
            esm = [
                persist.tile([128, NN], F32, tag=f"esm{m}", name=f"esm{m}")
                for m in range(MT)
            ]

            def prep_dma(g):
                for r in range(g * 8, g * 8 + 8):
                    nc.sync.dma_start(
                        out=zfull[:, r, :], in_=z[r * 128 : (r + 1) * 128, :]
                    )

            def prep_load(g):
                """squared norms + inv + normalized rows for group g."""
                for r in range(g * 8, g * 8 + 8):
                    tr = trashp.tile([128, NNW], BF16, tag="trash")
                    nc.scalar.activation(
                        out=tr[:, :512],
                        in_=zfull[:, r, :],
                        func=AF.Square,
                        accum_out=ssqall[:, r : r + 1],
                    )
                gs = slice(g * 8, g * 8 + 8)
                nc.scalar.sqrt(nrmall[:, gs], ssqall[:, gs])
                nc.vector.reciprocal(invall[:, gs], nrmall[:, gs])
                for r in range(g * 8, g * 8 + 8):
                    nrow = nrows.tile([128, 512], BF16, tag="nrow")
                    nc.vector.tensor_scalar_mul(
                        nrow, zfull[:, r, :], invall[:, r : r + 1]
                    )
                    yield r, nrow

            def prep_transpose(r, nrow):
                pstr = pstrp.tile([128, KT, 128], BF16, tag="pstr")
                for k in range(KT):
                    nc.tensor.transpose(
                        pstr[:, k, :], nrow[:, k * 128 : (k + 1) * 128], identS
                    )
                nc.vector.tensor_copy(
                    out=repsT[:, :, r * 128 : (r + 1) * 128], in_=pstr
                )

            def main_group(g):
                """column super-tile g of the sim block, all m."""
                for m in range(MT):
                    ps = mmp.tile([128, NNW], F32, tag="ps")
                    for h in (0, 1):
                        for k in range(KT):
                            nc.tensor.matmul(
                                ps[:, h * 512 : (h + 1) * 512],
                                lhsT=repsT[:, k, m * 128 : (m + 1) * 128],
                                rhs=repsT[
                                    :, k, g * NNW + h * 512 : g * NNW + (h + 1) * 512
                                ],
                                start=(k == 0),
                                stop=(k == KT - 1),
                            )
                    if g == 0:
                        # mask self-similarity: sim[p, m*128+p] -= 1e30
                        nc.vector.tensor_add(
                            ps[:, m * 128 : (m + 1) * 128],
                            ps[:, m * 128 : (m + 1) * 128],
                            negeyeS,
                        )
                    et = etrashp.tile([128, NNW], BF16, tag="etrash")
                    nc.scalar.activation(
                        out=et,
                        in_=ps,
                        func=AF.Exp,
                        bias=negfour,
                        scale=4.0,
                        accum_out=esm[m][:, g : g + 1],
                    )
                    nc.vector.reduce_max(maxm[m][:, g : g + 1], et, axis=AX.X)
                # positives (raw bf16 dots of rolled rows q, q+32), two per
                # group once their partner group is resident
                if g >= 4:
                    for q in (2 * (g - 4), 2 * (g - 4) + 1):
                        prod = nrows.tile([128, 512], F32, tag="prod")
                        nc.vector.tensor_mul(
                            prod, zfull[:, q, :], zfull[:, q + 32, :]
                        )
                        nc.vector.reduce_sum(posdt[:, q : q + 1], prod, axis=AX.X)

            # ---- software-pipelined schedule: prep one group ahead,
            # DMA two groups ahead ----
            prep_dma(0)
            prep_dma(1)
            for r, nrow in prep_load(0):
                prep_transpose(r, nrow)
            for g in range(NN):
                if g + 2 < NN:
                    prep_dma(g + 2)
                pending = list(prep_load(g + 1)) if g + 1 < NN else []
                main_group(g)
                for r, nrow in pending:
                    prep_transpose(r, nrow)

            for m in range(MT):
                nc.sync.dma_start(out=maxc_d[m], in_=maxm[m])
                nc.sync.dma_start(out=esum_d[m], in_=esm[m])
            nc.sync.dma_start(out=posd_d, in_=posdt)
            nc.sync.dma_start(out=invn_d, in_=invall)

    nc.compile()
    _CACHE["nc"] = nc
    return nc


def _host_inputs(z_i, z_j):
    reps = np.concatenate(
        [np.asarray(z_i, np.float32), np.asarray(z_j, np.float32)], axis=0
    )
    zb = reps.astype(ml_dtypes.bfloat16)
    ident = np.eye(128, dtype=np.float32).astype(ml_dtypes.bfloat16)
    negeye = (np.eye(128, dtype=np.float32) * -1.0e30).astype(np.float32)
    in_maps = []
    for c in range(NCORES):
        zc = np.ascontiguousarray(np.roll(zb, -c * NLOC, axis=0))
        in_maps.append({"z": zc, "ident": ident, "negeye": negeye})
    return in_maps


def _combine(results):
    pos = np.zeros(N, np.float64)
    hn = np.zeros(N, np.float64)
    S = 0.0
    for c, o in enumerate(results):
        maxc = np.asarray(o["maxc"], np.float64)   # [MT, 128, NN]
        esum = np.asarray(o["esum"], np.float64)   # [MT, 128, NN]
        posd = np.asarray(o["posd"], np.float64)   # [128, MT]
        invn = np.asarray(o["invn"], np.float64)   # [128, RT]
        # maxc holds max over exp(4*sim-4) per column super-tile (bf16
        # rounded); invert the exp to recover the sim max.
        hn_loc = (np.log(maxc.max(axis=2).reshape(NLOC)) + 4.0) / 4.0
        S += esum.sum()                            # self terms exp'd to 0
        invrow = invn.T.reshape(N)                 # rolled row index
        posl = posd.T.reshape(NLOC) * invrow[:NLOC] * invrow[B : B + NLOC]
        gl = (np.arange(NLOC) + c * NLOC) % N
        pos[gl] = posl
        hn[gl] = hn_loc
    ce = np.mean(np.logaddexp(0.0, 40.0 * hn - 20.0 * pos))
    npairs = N * (N - 1) // 2
    uniformity = np.log(S / 2.0 / npairs)
    return np.array(ce + 0.2 * uniformity, dtype=np.float32)


def run(z_i, z_j, **spmd_kwargs):
    nc = _build_program()
    in_maps = _host_inputs(z_i, z_j)
    res = run_bass_kernel_spmd(nc, in_maps, core_ids=list(range(NCORES)), **spmd_kwargs)
    return _combine(res.results), res


def kernel(z_i, z_j):
    loss, _ = run(z_i, z_j)
    return loss
